# revision 1
# baseline (speedup 1.0000x reference)
"""Grouped MoE MLP (SwiGLU) for TRN2, expert-parallel across 8 NeuronCores.

Problem: T=8192 tokens pre-permuted into 8 contiguous expert segments of 1024,
H=1024, I=2816, per-expert weights gate/up [H,I], down [I,H].
    o1 = x @ gate; o2 = x @ up; h = silu(o1)*o2; out = h @ down

Sharding: expert-parallel — core e computes expert e's segment entirely
(zero collectives). Host slices inputs per expert and concatenates outputs.

Device kernel layout (per core), all matmuls in float32r (tf32-like, full
1-cycle/row rate at N=512, ~1e-4 rel err):
  - x fed host-transposed: xt [H, TE] so H (contraction) is on partitions.
  - stage 1: per I-slab of 128: o1T/o2T [128i, TE] = gate/up-slab.T @ xt,
    PSUM-accumulated over 8 H-chunks; SwiGLU fused: hT = silu(o1T)*o2T
    kept in SBUF ([I, TE], 22 slabs).
  - stage 2: out[TE, H] = hT.T @ down, PSUM-accumulated over 22 I-slabs,
    per (m-tile 128 tokens, h-chunk 512).

Weights are host-rearranged so every DMA is a fully-contiguous block.
"""

import os
import numpy as np
from contextlib import ExitStack

E, H, I, T = 8, 1024, 2816, 8192
TE = T // E  # tokens per expert = 1024
KC = H // 128  # 8 h-chunks
IS = I // 128  # 22 i-slabs
NCH = 512  # moving free dim per matmul (one PSUM bank of fp32)

_cache: dict = {}


def _build_nc(dt_tag: str):
    from concourse import bacc
    import concourse.tile as tile
    import concourse.mybir as mybir
    from concourse.bass import ts

    f32 = mybir.dt.float32
    dt = {"f32r": mybir.dt.float32r, "bf16": mybir.dt.bfloat16}[dt_tag]

    nc = bacc.Bacc("TRN2", target_bir_lowering=False, debug=False, num_devices=8)
    xt_d = nc.dram_tensor("xt", [KC, 128, TE], dt, kind="ExternalInput").ap()
    gate_d = nc.dram_tensor("gate", [IS, 128, KC, 128], dt, kind="ExternalInput").ap()
    up_d = nc.dram_tensor("up", [IS, 128, KC, 128], dt, kind="ExternalInput").ap()
    down_d = nc.dram_tensor("down", [IS, 128, H], dt, kind="ExternalInput").ap()
    out_d = nc.dram_tensor("out", [TE, H], f32, kind="ExternalOutput").ap()

    silu_fn = mybir.ActivationFunctionType.Silu

    with tile.TileContext(nc) as tc, ExitStack() as ctx:
        xt_pool = ctx.enter_context(tc.tile_pool(name="xt", bufs=2 * KC))
        g_pool = ctx.enter_context(tc.tile_pool(name="g", bufs=2))
        u_pool = ctx.enter_context(tc.tile_pool(name="u", bufs=2))
        h_pool = ctx.enter_context(tc.tile_pool(name="h", bufs=IS))
        d_pool = ctx.enter_context(tc.tile_pool(name="d", bufs=IS + 1))
        s_pool = ctx.enter_context(tc.tile_pool(name="s", bufs=2))
        o_pool = ctx.enter_context(tc.tile_pool(name="o", bufs=2))
        ps1 = ctx.enter_context(tc.tile_pool(name="ps1", bufs=2, space="PSUM"))
        ps3 = ctx.enter_context(tc.tile_pool(name="ps3", bufs=2, space="PSUM"))

        # resident xt half-tiles [128h, NCH], split by t-chunk so the first
        # matmuls' dependencies are small; DMA emission order puts slab-0
        # gate + the tc0 halves first to minimize PE startup latency.
        gs, us = {}, {}
        # slab-0 gate/up split into two half-DMAs so the very first matmuls
        # are gated on ~256KB, not 512KB
        # wave 1 (exactly 8 DMAs -> 8 queues): g0 first half, xt tc0 halves
        # k=0..5, g0 second half. Covers MMs 0-5 plus the k>=4 gate tiles.
        gs[0] = g_pool.tile([128, KC, 128], dt, tag="g", name="g0")
        nc.sync.dma_start(out=gs[0][:, 0 : KC // 2, :], in_=gate_d[0, :, 0 : KC // 2])
        xth = [[None] * KC for _ in range(TE // NCH)]
        for k in range(6):
            t = xt_pool.tile([128, NCH], dt, tag="xt", name=f"xt0_{k}")
            nc.sync.dma_start(out=t[:], in_=xt_d[k, :, ts(0, NCH)])
            xth[0][k] = t
        nc.sync.dma_start(
            out=gs[0][:, KC // 2 : KC, :], in_=gate_d[0, :, KC // 2 : KC]
        )
        # wave 2: last two xt tc0 halves, u0 halves, then xt tc1 halves
        for k in (6, 7):
            t = xt_pool.tile([128, NCH], dt, tag="xt", name=f"xt0_{k}")
            nc.sync.dma_start(out=t[:], in_=xt_d[k, :, ts(0, NCH)])
            xth[0][k] = t
        us[0] = u_pool.tile([128, KC, 128], dt, tag="u", name="u0")
        nc.sync.dma_start(out=us[0][:, 0 : KC // 2, :], in_=up_d[0, :, 0 : KC // 2])
        nc.sync.dma_start(out=us[0][:, KC // 2 : KC, :], in_=up_d[0, :, KC // 2 : KC])
        for k in range(KC):
            t = xt_pool.tile([128, NCH], dt, tag="xt", name=f"xt1_{k}")
            nc.sync.dma_start(out=t[:], in_=xt_d[k, :, ts(1, NCH)])
            xth[1][k] = t

        # stage 1+2: per i-slab, o1T/o2T then fused SwiGLU into resident hT
        hts = []
        for i in range(IS):
            if i not in gs:
                gs[i] = g_pool.tile([128, KC, 128], dt, tag="g", name=f"g{i}")
                nc.sync.dma_start(out=gs[i][:], in_=gate_d[i])
                us[i] = u_pool.tile([128, KC, 128], dt, tag="u", name=f"u{i}")
                nc.sync.dma_start(out=us[i][:], in_=up_d[i])
            g, u = gs[i], us[i]
            ht = h_pool.tile([128, TE], dt, tag="h")
            for tci in range(TE // NCH):
                p1 = ps1.tile([128, NCH], f32, tag="p1")
                p2 = ps1.tile([128, NCH], f32, tag="p2")
                for k in range(KC):
                    nc.tensor.matmul(
                        p1[:],
                        lhsT=g[:, k, :],
                        rhs=xth[tci][k][:],
                        start=(k == 0),
                        stop=(k == KC - 1),
                    )
                for k in range(KC):
                    nc.tensor.matmul(
                        p2[:],
                        lhsT=u[:, k, :],
                        rhs=xth[tci][k][:],
                        start=(k == 0),
                        stop=(k == KC - 1),
                    )
                sl = s_pool.tile([128, NCH], f32, tag="s")
                nc.scalar.activation(sl[:], p1[:], silu_fn)
                nc.vector.tensor_mul(ht[:, ts(tci, NCH)], sl[:], p2[:])
            hts.append(ht)

        # stage 3: out[m,hc] = sum_i hT_i[:, m].T @ down_i[:, hc]
        # hc=1's first 16 d-tiles go in the xt tag: xt tiles are dead after
        # stage 1, so those slots free mid-stage-3-hc0 and the hc=1 loads
        # prefetch instead of stalling on d-slot releases at the hc boundary.
        for hc in range(H // NCH):
            dts = []
            for i in range(IS):
                pool_tag = "xt" if (hc == 1 and i < 2 * KC) else "d"
                d = d_pool.tile(
                    [128, NCH], dt, tag=pool_tag, name=f"d{hc}_{i}"
                ) if pool_tag == "d" else xt_pool.tile(
                    [128, NCH], dt, tag="xt", name=f"d{hc}_{i}"
                )
                nc.sync.dma_start(out=d[:], in_=down_d[i, :, ts(hc, NCH)])
                dts.append(d)
            for m in range(TE // 128):
                po = ps3.tile([128, NCH], f32, tag="po")
                for i in range(IS):
                    nc.tensor.matmul(
                        po[:],
                        lhsT=hts[i][:, ts(m, 128)],
                        rhs=dts[i][:],
                        start=(i == 0),
                        stop=(i == IS - 1),
                    )
                ob = o_pool.tile([128, NCH], f32, tag="o")
                nc.vector.tensor_copy(ob[:], po[:])
                nc.scalar.dma_start(out=out_d[ts(m, 128), ts(hc, NCH)], in_=ob[:])

    nc.compile()
    return nc


def _get_nc(dt_tag: str):
    if dt_tag not in _cache:
        _cache[dt_tag] = _build_nc(dt_tag)
    return _cache[dt_tag]


def _prep_in_maps(x, gate, up, down, dt_tag: str):
    """Slice per expert and rearrange for contiguous device DMAs."""
    cast = None
    if dt_tag == "bf16":
        import ml_dtypes

        cast = ml_dtypes.bfloat16
    in_maps = []
    for e in range(E):
        xe = np.ascontiguousarray(x[e * TE : (e + 1) * TE].T)  # [H, TE]
        # gate/up [H, I] -> [IS, 128p(h%128), KC(h//128), 128m(i%128)]
        ge = np.ascontiguousarray(
            gate[e].reshape(KC, 128, IS, 128).transpose(2, 1, 0, 3)
        )
        ue = np.ascontiguousarray(up[e].reshape(KC, 128, IS, 128).transpose(2, 1, 0, 3))
        de = down[e].reshape(IS, 128, H)
        if cast is not None:
            xe, ge, ue, de = (a.astype(cast) for a in (xe, ge, ue, de))
        in_maps.append(
            {
                "xt": xe.reshape(KC, 128, TE),
                "gate": ge,
                "up": ue,
                "down": np.ascontiguousarray(de),
            }
        )
    return in_maps


def run(inputs: dict, trace: bool = False, tmpdir=None, dt_tag=None):
    """Full-input entry. Returns (output [T,H] f32, BassKernelResults|None)."""
    x = np.asarray(inputs["permuted_local_hidden_states"], dtype=np.float32)
    gate = np.asarray(inputs["grouped_gate_proj"], dtype=np.float32)
    up = np.asarray(inputs["grouped_up_proj"], dtype=np.float32)
    down = np.asarray(inputs["grouped_down_proj"], dtype=np.float32)
    tpe = np.asarray(inputs["tokens_per_expert"]).astype(np.int64)

    if not (x.shape == (T, H) and tpe.shape == (E,) and np.all(tpe == TE)):
        # general ragged fallback (host): correctness-only path
        out = np.empty((x.shape[0], down.shape[2]), dtype=np.float32)
        off = 0
        for e in range(E):
            n = int(tpe[e])
            xe = x[off : off + n]
            o1 = xe @ gate[e]
            o2 = xe @ up[e]
            with np.errstate(over="ignore"):
                hgl = (o1 / (1.0 + np.exp(-o1))) * o2
            out[off : off + n] = hgl @ down[e]
            off += n
        return out, None

    dt_tag = dt_tag or os.environ.get("BASS_MOE_DT", "f32r")
    from concourse.bass_utils import run_bass_kernel_spmd

    nc = _get_nc(dt_tag)
    in_maps = _prep_in_maps(x, gate, up, down, dt_tag)
    res = run_bass_kernel_spmd(
        nc, in_maps, list(range(E)), trace=trace, tmpdir=tmpdir
    )
    out = np.concatenate([res.results[e]["out"] for e in range(E)], axis=0)
    return out, res


def kernel(**inputs) -> np.ndarray:
    out, _ = run(inputs, trace=False)
    return out



# revision 2
# speedup vs baseline: 1.0066x; 1.0066x over previous
"""Grouped MoE MLP (SwiGLU) for TRN2, expert-parallel across 8 NeuronCores.

Problem: T=8192 tokens pre-permuted into 8 contiguous expert segments of 1024,
H=1024, I=2816, per-expert weights gate/up [H,I], down [I,H].
    o1 = x @ gate; o2 = x @ up; h = silu(o1)*o2; out = h @ down

Sharding: expert-parallel - core e computes expert e's segment entirely
(zero collectives). Host slices inputs per expert and concatenates outputs.

Device kernel (per core), all matmuls in bf16 (rel err ~4e-3, budget 2e-2):
  - x host-transposed + packed: xt [2(tc), 128, KC, 512] so H (contraction)
    is on partitions; one resident SBUF tile [128, KC*512] per 512-token
    chunk.
  - gate+up packed per i-slab: gu [IS, 128, 2, KC, 128] -> one DMA per slab.
  - stage 1: per i-slab of 128: o1T/o2T [128i, 512] = slab.T @ xt chunks,
    PSUM-accumulated over 8 h-chunks; SwiGLU fused: hT = silu(o1T)*o2T
    kept resident in SBUF ([I, TE] bf16, 22 slabs).
  - stage 2: out[TE, H] = hT.T @ down, PSUM-accumulated over 22 i-slabs,
    per (m-tile 128 tokens, h-chunk 512). Output stored bf16, host upcasts.
  - first-wave DMAs are issued from sync+scalar+gpsimd in parallel (each
    engine issues a DMA in ~0.6us serially; 3 engines keep the PE fed from
    ~9.5us), later weights stream on sync while scalar runs SwiGLU.
  - last output tile is split in half so the final PSUM->SBUF->HBM flush
    tail is short.
"""

import os
import numpy as np
from contextlib import ExitStack

E, H, I, T = 8, 1024, 2816, 8192
TE = T // E  # tokens per expert = 1024
KC = H // 128  # 8 h-chunks
IS = I // 128  # 22 i-slabs
NCH = 512  # moving free dim per matmul (one PSUM bank of fp32)

_cache: dict = {}


def _build_nc(dt_tag: str):
    from concourse import bacc
    import concourse.tile as tile
    import concourse.mybir as mybir
    from concourse.bass import ts

    f32 = mybir.dt.float32
    dt = {"f32r": mybir.dt.float32r, "bf16": mybir.dt.bfloat16}[dt_tag]

    nc = bacc.Bacc("TRN2", target_bir_lowering=False, debug=False, num_devices=8)
    xt_d = nc.dram_tensor("xt", [2, 128, KC, NCH], dt, kind="ExternalInput").ap()
    gu_d = nc.dram_tensor("gu", [IS, 128, 2, KC, 128], dt, kind="ExternalInput").ap()
    down_d = nc.dram_tensor("down", [IS, 128, H], dt, kind="ExternalInput").ap()
    out_d = nc.dram_tensor("out", [TE, H], dt, kind="ExternalOutput").ap()

    silu_fn = mybir.ActivationFunctionType.Silu

    with tile.TileContext(nc) as tc, ExitStack() as ctx:
        xt_pool = ctx.enter_context(tc.tile_pool(name="xt", bufs=2))
        gu_pool = ctx.enter_context(tc.tile_pool(name="gu", bufs=5))
        h_pool = ctx.enter_context(tc.tile_pool(name="h", bufs=IS))
        d_pool = ctx.enter_context(tc.tile_pool(name="d", bufs=2 * IS))
        s_pool = ctx.enter_context(tc.tile_pool(name="s", bufs=2))
        o_pool = ctx.enter_context(tc.tile_pool(name="o", bufs=3))
        ps1 = ctx.enter_context(tc.tile_pool(name="ps1", bufs=2, space="PSUM"))
        ps3 = ctx.enter_context(tc.tile_pool(name="ps3", bufs=3, space="PSUM"))

        # resident x chunks: [128, KC, 512] per 512-token chunk
        xtall = [
            xt_pool.tile([128, KC, NCH], dt, tag="xt", name=f"xt{tci}")
            for tci in range(2)
        ]
        gus = {0: gu_pool.tile([128, 2, KC, 128], dt, tag="gu", name="gu0")}
        g0 = gus[0]

        # ---- wave 1: interleave issue across sync/scalar/gpsimd so the
        # PE is fed at ~1 tile/0.2us despite ~0.6us serial issue per engine.
        # Consumption order: g0(k0..7) w/ xt0(k0..7), u0(k0..7), tc1 repeat.
        nc.sync.dma_start(out=g0[:, 0, 0:4, :], in_=gu_d[0, :, 0, 0:4])  # g0 k0-3
        nc.scalar.dma_start(out=xtall[0][:, 0, :], in_=xt_d[0, :, 0])  # xt00
        nc.gpsimd.dma_start(out=xtall[0][:, 1, :], in_=xt_d[0, :, 1])  # xt01
        nc.sync.dma_start(out=g0[:, 0, 4:8, :], in_=gu_d[0, :, 0, 4:8])  # g0 k4-7
        nc.scalar.dma_start(out=xtall[0][:, 2, :], in_=xt_d[0, :, 2])  # xt02
        nc.gpsimd.dma_start(out=xtall[0][:, 3, :], in_=xt_d[0, :, 3])  # xt03
        nc.sync.dma_start(out=xtall[0][:, 4, :], in_=xt_d[0, :, 4])  # xt04
        nc.scalar.dma_start(out=xtall[0][:, 5, :], in_=xt_d[0, :, 5])  # xt05
        nc.gpsimd.dma_start(out=xtall[0][:, 6, :], in_=xt_d[0, :, 6])  # xt06
        nc.scalar.dma_start(out=xtall[0][:, 7, :], in_=xt_d[0, :, 7])  # xt07
        nc.sync.dma_start(out=g0[:, 1, 0:4, :], in_=gu_d[0, :, 1, 0:4])  # u0 k0-3
        nc.scalar.dma_start(out=g0[:, 1, 4:8, :], in_=gu_d[0, :, 1, 4:8])  # u0 k4-7
        nc.gpsimd.dma_start(out=xtall[1][:, 0:4, :], in_=xt_d[1, :, 0:4])  # xt1 a
        nc.scalar.dma_start(out=xtall[1][:, 4:8, :], in_=xt_d[1, :, 4:8])  # xt1 b

        # bulk weight stream on sync: remaining gu slabs (slot-gated by
        # gu_pool bufs) with all down tiles interleaved into the idle waits.
        dts = [[None] * IS for _ in range(2)]

        def emit_d(hc, i):
            d = d_pool.tile([128, NCH], dt, tag="d", name=f"d{hc}_{i}")
            nc.sync.dma_start(out=d[:], in_=down_d[i, :, ts(hc, NCH)])
            dts[hc][i] = d

        dq = [(hc, i) for hc in range(2) for i in range(IS)]
        dqi = 0
        for i in range(1, IS):
            gus[i] = gu_pool.tile([128, 2, KC, 128], dt, tag="gu", name=f"gu{i}")
            nc.sync.dma_start(out=gus[i][:], in_=gu_d[i])
            while dqi < min(len(dq), 2 * i + 2):
                emit_d(*dq[dqi])
                dqi += 1
        while dqi < len(dq):
            emit_d(*dq[dqi])
            dqi += 1

        # stage 1: per i-slab, o1T/o2T then fused SwiGLU into resident hT
        hts = []
        for i in range(IS):
            gu = gus[i]
            ht = h_pool.tile([128, TE], dt, tag="h")
            for tci in range(TE // NCH):
                p1 = ps1.tile([128, NCH], f32, tag="p1")
                p2 = ps1.tile([128, NCH], f32, tag="p2")
                for k in range(KC):
                    nc.tensor.matmul(
                        p1[:],
                        lhsT=gu[:, 0, k, :],
                        rhs=xtall[tci][:, k, :],
                        start=(k == 0),
                        stop=(k == KC - 1),
                    )
                for k in range(KC):
                    nc.tensor.matmul(
                        p2[:],
                        lhsT=gu[:, 1, k, :],
                        rhs=xtall[tci][:, k, :],
                        start=(k == 0),
                        stop=(k == KC - 1),
                    )
                sl = s_pool.tile([128, NCH], f32, tag="s")
                nc.scalar.activation(sl[:], p1[:], silu_fn)
                nc.vector.tensor_mul(ht[:, ts(tci, NCH)], sl[:], p2[:])
            hts.append(ht)

        # stage 2: out[m,hc] = sum_i hT_i[:, m].T @ down_i[:, hc]
        # last tile split in half to shorten the final flush tail
        for hc in range(H // NCH):
            for m in range(TE // 128):
                last = hc == H // NCH - 1 and m == TE // 128 - 1
                if not last:
                    po = ps3.tile([128, NCH], f32, tag="po")
                    for i in range(IS):
                        nc.tensor.matmul(
                            po[:],
                            lhsT=hts[i][:, ts(m, 128)],
                            rhs=dts[hc][i][:],
                            start=(i == 0),
                            stop=(i == IS - 1),
                        )
                    ob = o_pool.tile([128, NCH], dt, tag="o")
                    nc.vector.tensor_copy(ob[:], po[:])
                    nc.scalar.dma_start(
                        out=out_d[ts(m, 128), ts(hc, NCH)], in_=ob[:]
                    )
                else:
                    for q in range(2):
                        po = ps3.tile([128, NCH], f32, tag="po")
                        for i in range(IS):
                            nc.tensor.matmul(
                                po[:, 0:256],
                                lhsT=hts[i][:, ts(m, 128)],
                                rhs=dts[hc][i][:, ts(q, 256)],
                                start=(i == 0),
                                stop=(i == IS - 1),
                            )
                        ob = o_pool.tile([128, 256], dt, tag="oh", bufs=2)
                        nc.vector.tensor_copy(ob[:], po[:, 0:256])
                        nc.scalar.dma_start(
                            out=out_d[
                                ts(m, 128),
                                hc * NCH + q * 256 : hc * NCH + (q + 1) * 256,
                            ],
                            in_=ob[:],
                        )

    nc.compile()
    return nc


def _get_nc(dt_tag: str):
    if dt_tag not in _cache:
        _cache[dt_tag] = _build_nc(dt_tag)
    return _cache[dt_tag]


def _to_bf16(a: np.ndarray) -> np.ndarray:
    """Fast float32 -> bfloat16 with round-to-nearest-even."""
    import ml_dtypes

    u = a.view(np.uint32)
    r = ((u >> 16) & 1) + np.uint32(0x7FFF)
    return ((u + r) >> 16).astype(np.uint16).view(ml_dtypes.bfloat16)


def _prep_in_maps(x, gate, up, down, dt_tag: str):
    """Slice per expert and rearrange for contiguous device DMAs."""
    in_maps = []
    for e in range(E):
        xe = x[e * TE : (e + 1) * TE]  # [TE, H]
        # [2(tc), 128(h%128), KC(h//128), 512(t%512)]
        xtp = np.ascontiguousarray(
            xe.T.reshape(KC, 128, 2, NCH).transpose(2, 1, 0, 3)
        )
        # gate/up [H, I] -> [IS, 128(h%128), KC(h//128), 128(i%128)]
        ge = gate[e].reshape(KC, 128, IS, 128).transpose(2, 1, 0, 3)
        ue = up[e].reshape(KC, 128, IS, 128).transpose(2, 1, 0, 3)
        gue = np.ascontiguousarray(np.stack([ge, ue], axis=2))
        de = np.ascontiguousarray(down[e].reshape(IS, 128, H))
        if dt_tag == "bf16":
            xtp, gue, de = (_to_bf16(a) for a in (xtp, gue, de))
        in_maps.append({"xt": xtp, "gu": gue, "down": de})
    return in_maps


def run(inputs: dict, trace: bool = False, tmpdir=None, dt_tag=None):
    """Full-input entry. Returns (output [T,H] f32, BassKernelResults|None)."""
    x = np.asarray(inputs["permuted_local_hidden_states"], dtype=np.float32)
    gate = np.asarray(inputs["grouped_gate_proj"], dtype=np.float32)
    up = np.asarray(inputs["grouped_up_proj"], dtype=np.float32)
    down = np.asarray(inputs["grouped_down_proj"], dtype=np.float32)
    tpe = np.asarray(inputs["tokens_per_expert"]).astype(np.int64)

    if not (x.shape == (T, H) and tpe.shape == (E,) and np.all(tpe == TE)):
        # general ragged fallback (host): correctness-only path
        out = np.empty((x.shape[0], down.shape[2]), dtype=np.float32)
        off = 0
        for e in range(E):
            n = int(tpe[e])
            xe = x[off : off + n]
            o1 = xe @ gate[e]
            o2 = xe @ up[e]
            with np.errstate(over="ignore"):
                hgl = (o1 / (1.0 + np.exp(-o1))) * o2
            out[off : off + n] = hgl @ down[e]
            off += n
        return out, None

    dt_tag = dt_tag or os.environ.get("BASS_MOE_DT", "bf16")
    from concourse.bass_utils import run_bass_kernel_spmd

    nc = _get_nc(dt_tag)
    in_maps = _prep_in_maps(x, gate, up, down, dt_tag)
    res = run_bass_kernel_spmd(
        nc, in_maps, list(range(E)), trace=trace, tmpdir=tmpdir
    )
    out = np.concatenate(
        [np.asarray(res.results[e]["out"], dtype=np.float32) for e in range(E)],
        axis=0,
    )
    return out, res


def kernel(**inputs) -> np.ndarray:
    out, _ = run(inputs, trace=False)
    return out
